# revision 9
# baseline (speedup 1.0000x reference)
"""ONI-Norm TRN2 kernel v6: bf16 end-to-end, DMA-floor-targeted schedule.

Math per group g (128 rows, fan_in K=18432):
  Zc = Z - mean(Z, axis=1)
  S  = Zc Zc^T + eps I;  S /= ||S||_F
  B via 5 Newton-Schulz iters;  W = (B Zc) / sqrt(||S||_F)

Measured constraints this schedule is built around:
  * DMA: ~270 GB/s/core sustained under 8-core SPMD -> bf16 in+out
    (18.9 MB) floors the kernel at ~70us. Total traffic is kept at the
    minimum; descriptors are 12KB (loads) / 4KB (stores).
  * PE: 128-wide matmuls/transposes are LDWEIGHTS-paced (~70ns);
    512-wide proj matmuls are stream-paced but HAM-throttled 2x if the
    PE idles -> the schedule keeps the PE continuously fed (lazy
    mean-chain PE ops, proj(0) woven with gram(1) tail, transpose
    fillers in the proj(1) tail).
  * DVE 2x mode applies only to all-2-byte ops -> zt copies (bf16
    PSUM->SBUF) go to DVE; fp32-sourced output copies split DVE/ACT.
  * row sums ride the Gram matmuls as a 129th ones-column.
"""

import math
from contextlib import ExitStack

import numpy as np
from ml_dtypes import bfloat16

import concourse.bacc as bacc
import concourse.mybir as mybir
from concourse.bass import ds, ts, MemorySpace
from concourse.bass_isa import ReduceOp
from concourse.bass_utils import run_bass_kernel_spmd
from concourse.tile import TileContext

P = 128
K = 18432
G_TOTAL = 16
N_CORES = 8
G_PER_CORE = G_TOTAL // N_CORES
ROWS_PER_CORE = G_PER_CORE * P
T_NS = 5
EPS = 1e-5

BF16 = mybir.dt.bfloat16
F32 = mybir.dt.float32

LOAD_CHUNK = 6144           # 12KB descriptors, 3 loads per group
SLAB = 2048                 # transpose slab: 16 blocks -> one 2-bank psum
SBLK = SLAB // P            # 16
N_SLABS = K // SLAB         # 9 per group
ZTW = SBLK * (P + 1)        # 2064
PROJ_U = 1024               # proj psum unit (2 banks)
N_UNITS = K // PROJ_U       # 18 per group
OUT_CHUNK = 2048            # store granularity (2 units)


def build_nc():
    nc = bacc.Bacc("TRN2", target_bir_lowering=False)
    x = nc.dram_tensor("x", [ROWS_PER_CORE, K], BF16, kind="ExternalInput")
    y = nc.dram_tensor("y", [ROWS_PER_CORE, K], BF16, kind="ExternalOutput")

    with TileContext(nc) as tc, ExitStack() as ctx:
        consts = ctx.enter_context(tc.tile_pool(name="consts", bufs=1))

        def diag_const(dtype, fill, tag):
            t = consts.tile([P, P], dtype, tag=tag, name=tag)
            nc.gpsimd.memset(t, 0.0)
            nc.gpsimd.affine_select(
                out=t, in_=t, compare_op=mybir.AluOpType.not_equal,
                fill=fill, base=0, pattern=[[-1, P]], channel_multiplier=1,
            )
            return t

        id_bf = diag_const(BF16, 1.0, "idbf")
        eye15 = diag_const(BF16, 1.5, "eye15")
        eps_eye = diag_const(F32, EPS, "epseye")
        ones_bf = consts.tile([P, P], BF16, tag="onesbf")
        nc.gpsimd.memset(ones_bf, 1.0)

        zpool = ctx.enter_context(tc.tile_pool(name="z", bufs=2 * 3))
        ztp = ctx.enter_context(tc.tile_pool(name="zt", bufs=3))
        outp = ctx.enter_context(tc.tile_pool(name="out", bufs=4))
        nsp = ctx.enter_context(tc.tile_pool(name="ns", bufs=2))
        vecp = ctx.enter_context(tc.tile_pool(name="vec", bufs=2))
        # PSUM: "big" 4KB slots shared by transpose tiles ([128,2048] bf16)
        # and proj tiles ([128,1024] f32): 2 bufs = 4 banks.
        ps_big = ctx.enter_context(tc.tile_pool(name="psB", bufs=2, space=MemorySpace.PSUM))
        ps_S = ctx.enter_context(tc.tile_pool(name="psS", bufs=2, space=MemorySpace.PSUM))
        ps_ns = ctx.enter_context(tc.tile_pool(name="psN", bufs=2, space=MemorySpace.PSUM))

        st = [dict() for _ in range(G_PER_CORE)]

        def emit_loads(g, split_first=False):
            s = st[g]
            s["zs"] = []
            for c in range(3):
                z = zpool.tile([P, LOAD_CHUNK], BF16, tag="z", name=f"z{g}_{c}")
                if split_first and c == 0:
                    for t3 in range(3):
                        nc.sync.dma_start(
                            z[:, ts(t3, LOAD_CHUNK // 3)],
                            x[ds(g * P, P),
                              ds(t3 * (LOAD_CHUNK // 3), LOAD_CHUNK // 3)],
                        )
                else:
                    nc.sync.dma_start(z, x[ds(g * P, P), ts(c, LOAD_CHUNK)])
                s["zs"].append(z)

        def z_cols(g, col, width):
            c, o = divmod(col, LOAD_CHUNK)
            assert o + width <= LOAD_CHUNK
            return st[g]["zs"][c][:, ds(o, width)]

        def emit_zt(g, si):
            """16 PE transposes of slab si -> 2-bank psum -> one DVE copy
            into the ones-interleaved zt tile."""
            s = st[g]
            tp = ps_big.tile([P, SLAB], BF16, tag="big", name=f"tp{g}_{si}")
            for b in range(SBLK):
                nc.tensor.transpose(
                    tp[:, ts(b, P)], z_cols(g, si * SLAB + b * P, P), id_bf,
                )
            zt = ztp.tile([P, ZTW], BF16, tag="zt", name=f"zt{g}_{si}")
            zv = zt.rearrange("p (b c) -> p b c", c=P + 1)
            nc.gpsimd.memset(zv[:, :, P:P + 1], 1.0)
            nc.vector.tensor_copy(zv[:, :, 0:P], tp)
            s.setdefault("zt", {})[si] = zt

        def emit_gram_slab(g, si):
            s = st[g]
            if si == 0:
                s["S_ps"] = ps_S.tile([P, P + 1], F32, tag="S", name=f"Sps{g}")
            zt = s["zt"].pop(si)
            for b in range(SBLK):
                nc.tensor.matmul(
                    s["S_ps"],
                    zt[:, ds(b * (P + 1), P)],
                    zt[:, ds(b * (P + 1), P + 1)],
                    start=(si == 0 and b == 0), stop=False,
                )

        _fill = [0]

        def emit_filler(n):
            """Dead PE transposes to hold HAM boost during copy-paced spans."""
            dead = ps_S.tile([P, P], BF16, tag="S", name=f"dead{_fill[0]}")
            for i in range(n):
                col = ((_fill[0] * 11 + i * 5) % (K // P)) * P
                nc.tensor.transpose(dead, z_cols(1, col, P), id_bf)
            _fill[0] += 1

        def emit_mean_A(g):
            s = st[g]
            r = vecp.tile([P, 1], F32, tag=f"r{g}", name=f"r{g}")
            nc.vector.tensor_copy(r, s["S_ps"][:, ds(P, 1)])
            s["r"] = r
            m12 = vecp.tile([P, 1], F32, tag=f"m12{g}", name=f"m12{g}")
            nc.vector.tensor_scalar_mul(m12, r, math.sqrt(K / P) / K)
            Mm = vecp.tile([P, P], BF16, tag=f"Mm{g}", name=f"Mm{g}")
            nc.vector.tensor_scalar_mul(Mm, ones_bf, m12)
            s["Mm"] = Mm

        def emit_mean_B(g):
            s = st[g]
            M_ps = ps_ns.tile([P, P], BF16, tag="ns", name=f"Mps{g}")
            nc.tensor.transpose(M_ps, s["Mm"], id_bf)
            Ma = vecp.tile([P, P + 1], BF16, tag=f"Ma{g}", name=f"Ma{g}")
            nc.gpsimd.memset(Ma[:, ds(P, 1)], 0.0)
            nc.vector.tensor_copy(Ma[:, ds(0, P)], M_ps)
            Mb = vecp.tile([P, P], BF16, tag=f"Mb{g}", name=f"Mb{g}")
            nc.vector.tensor_scalar_mul(Mb, Ma[:, ds(0, P)], -1.0)
            nc.tensor.matmul(s["S_ps"], Mb, Ma, start=False, stop=True)

        def emit_mean_C(g):
            s = st[g]
            S = nsp.tile([P, P], F32, tag="S", name=f"S{g}")
            nc.vector.scalar_tensor_tensor(
                S, s["S_ps"][:, ds(0, P)], 1.0, eps_eye,
                mybir.AluOpType.mult, mybir.AluOpType.add,
            )
            sq = nsp.tile([P, P], BF16, tag="sq", name=f"sq{g}")
            frob2 = vecp.tile([P, 1], F32, tag=f"fr{g}", name=f"fr{g}")
            nc.scalar.activation(
                sq, S, mybir.ActivationFunctionType.Square, accum_out=frob2
            )
            nc.gpsimd.partition_all_reduce(frob2, frob2, P, ReduceOp.add)
            nu = vecp.tile([P, 1], F32, tag=f"nu{g}", name=f"nu{g}")
            nc.scalar.sqrt(nu, frob2)
            nu2 = vecp.tile([P, 1], F32, tag=f"nu2{g}", name=f"nu2{g}")
            nc.vector.tensor_scalar_mul(nu2, nu, 2.0)
            inv_nu2 = vecp.tile([P, 1], F32, tag=f"inu{g}", name=f"inu{g}")
            nc.vector.reciprocal(inv_nu2, nu2)          # 1/(2 nu)
            inv_nu = vecp.tile([P, 1], F32, tag=f"invnu{g}", name=f"invnu{g}")
            nc.vector.tensor_scalar_mul(inv_nu, inv_nu2, 2.0)
            osc = vecp.tile([P, 1], F32, tag=f"osc{g}", name=f"osc{g}")
            nc.scalar.activation(osc, inv_nu, mybir.ActivationFunctionType.Sqrt)
            s["osc"] = osc
            Sh = nsp.tile([P, P], BF16, tag="Sh", name=f"Sh{g}")
            nc.scalar.activation(
                Sh, S, mybir.ActivationFunctionType.Identity, scale=inv_nu2
            )
            s["Sh"] = Sh
            B = nsp.tile([P, P], BF16, tag=f"B{g}", name=f"B0_{g}")
            nc.vector.tensor_sub(B, eye15, Sh)
            s["B"] = B

        def emit_ns_step(g, it, sub):
            s = st[g]
            if sub == 0:
                bb_ps = ps_ns.tile([P, P], F32, tag="ns", name=f"bb{g}_{it}")
                nc.tensor.matmul(bb_ps, s["B"], s["B"], start=True, stop=True)
                BB = nsp.tile([P, P], BF16, tag=f"BB{g}", name=f"BB{g}_{it}")
                nc.vector.tensor_copy(BB, bb_ps)
                s["BB"] = BB
            elif sub == 1:
                b3_ps = ps_ns.tile([P, P], F32, tag="ns", name=f"b3{g}_{it}")
                nc.tensor.matmul(b3_ps, s["BB"], s["B"], start=True, stop=True)
                B3 = nsp.tile([P, P], BF16, tag=f"B3{g}", name=f"B3_{g}_{it}")
                nc.vector.tensor_copy(B3, b3_ps)
                s["B3"] = B3
            else:
                p_ps = ps_ns.tile([P, P], F32, tag="ns", name=f"pp{g}_{it}")
                nc.tensor.matmul(p_ps, s["B3"], s["Sh"], start=True, stop=True)
                Bn = nsp.tile([P, P], BF16, tag=f"Bn{g}", name=f"Bn{g}_{it}")
                nc.vector.scalar_tensor_tensor(
                    Bn, s["B"], 1.5, p_ps,
                    mybir.AluOpType.mult, mybir.AluOpType.subtract,
                )
                s["B"] = Bn

        def emit_cbias(g):
            s = st[g]
            Bs = nsp.tile([P, P], BF16, tag=f"Bs{g}", name=f"Bs{g}")
            nc.vector.tensor_scalar_mul(Bs, s["B"], s["osc"])
            s["Bs"] = Bs
            mu = vecp.tile([P, 1], BF16, tag=f"mu{g}", name=f"mu{g}")
            nc.vector.tensor_scalar_mul(mu, s["r"], 1.0 / K)
            c_ps = ps_ns.tile([P, 1], F32, tag="ns", name=f"cps{g}")
            nc.tensor.matmul(c_ps, Bs, mu, start=True, stop=True)
            nbias = vecp.tile([P, 1], F32, tag=f"nb{g}", name=f"nb{g}")
            nc.vector.tensor_scalar_mul(nbias, c_ps, -1.0)
            s["nbias"] = nbias

        def emit_proj_unit(g, u, eng):
            s = st[g]
            c, t = divmod(u, OUT_CHUNK // PROJ_U)
            if t == 0:
                s["out_t"] = outp.tile(
                    [P, OUT_CHUNK], BF16, tag="out", name=f"o{g}_{c}"
                )
            pr = ps_big.tile([P, PROJ_U], F32, tag="big", name=f"pr{g}_{u}")
            for h in range(2):
                nc.tensor.matmul(
                    pr[:, ts(h, 512)], s["Bs"],
                    z_cols(g, u * PROJ_U + h * 512, 512),
                    start=True, stop=True,
                )
            dst = s["out_t"][:, ts(t, PROJ_U)]
            if eng == "d":
                nc.vector.tensor_scalar_add(dst, pr, s["nbias"])
            else:
                nc.scalar.activation(
                    dst, pr, mybir.ActivationFunctionType.Identity,
                    bias=s["nbias"],
                )
            if t == OUT_CHUNK // PROJ_U - 1:
                nc.sync.dma_start(y[ds(g * P, P), ts(c, OUT_CHUNK)], s["out_t"])

        # ---------------- emission schedule ----------------
        emit_loads(0, split_first=True)
        emit_loads(1)

        # B: gram(0) solid (PE: 9 slabs x 32 ldw-paced ops)
        emit_zt(0, 0)
        for si in range(N_SLABS):
            if si + 1 < N_SLABS:
                emit_zt(0, si + 1)
            emit_gram_slab(0, si)
        emit_mean_A(0)

        # C: gram(1) H1 (6 slabs) with mean_B/C(0) + NS(0) woven in
        H1 = 6
        ns_steps = [(it, sub) for it in range(T_NS - 1) for sub in range(3)]
        k = 0
        emit_zt(1, 0)
        for si in range(H1):
            if si + 1 < N_SLABS:
                emit_zt(1, si + 1)
            emit_gram_slab(1, si)
            if si == 0:
                emit_mean_B(0)
            elif si == 1:
                emit_mean_C(0)
            else:
                want = (si - 1) * len(ns_steps) // (H1 - 2)
                while k < want:
                    emit_ns_step(0, *ns_steps[k])
                    k += 1
        while k < len(ns_steps):
            emit_ns_step(0, *ns_steps[k])
            emit_filler(2)
            k += 1
        emit_cbias(0)

        # E: proj(0) units 0..11 weaving gram(1) H2 (slabs 6..8) + mean(1)
        oc = "dadadadadada"
        gsi = H1
        for u in range(12):
            emit_proj_unit(0, u, oc[u])
            if u % 4 == 0 and gsi < N_SLABS:
                if gsi + 1 < N_SLABS:
                    emit_zt(1, gsi + 1)
                emit_gram_slab(1, gsi)
                gsi += 1
            if u == 9:
                emit_mean_A(1)
            elif u == 10:
                emit_mean_B(1)
            elif u == 11:
                emit_mean_C(1)

        # F: proj(0) units 12..17 (copies mostly ACT), NS(1) on DVE
        k = 0
        oc = "aadaad"
        for i, u in enumerate(range(12, 18)):
            want = (i + 1) * len(ns_steps) // 6
            while k < want:
                emit_ns_step(1, *ns_steps[k])
                k += 1
            emit_proj_unit(0, u, oc[i])
        while k < len(ns_steps):
            emit_ns_step(1, *ns_steps[k])
            emit_filler(2)
            k += 1
        emit_cbias(1)

        # G: proj(1) with boost filler
        oc = "dadadadadadadadada"
        for u in range(N_UNITS):
            emit_proj_unit(1, u, oc[u])
            emit_filler(3)

    nc.finalize()
    return nc


_NC_CACHE = None


def _get_nc():
    global _NC_CACHE
    if _NC_CACHE is None:
        _NC_CACHE = build_nc()
    return _NC_CACHE


def kernel(weight, _trace=False):
    w = np.ascontiguousarray(np.asarray(weight, dtype=np.float32))
    assert w.shape == (G_TOTAL * P, K), w.shape
    wb = w.astype(bfloat16)
    nc = _get_nc()
    in_maps = [
        {"x": np.ascontiguousarray(wb[core * ROWS_PER_CORE:(core + 1) * ROWS_PER_CORE])}
        for core in range(N_CORES)
    ]
    res = run_bass_kernel_spmd(
        nc, in_maps, core_ids=list(range(N_CORES)), trace=_trace
    )
    out = np.concatenate(
        [np.asarray(r["y"]).astype(np.float32) for r in res.results], axis=0
    )
    if _trace:
        return out, res
    return out


# revision 10
# speedup vs baseline: 1.2285x; 1.2285x over previous
"""ONI-Norm TRN2 kernel v7: fp8 host-transposed Gram stream + bf16 proj.

Math per group g (128 rows, fan_in K=18432):
  Zc = Z - mean(Z, axis=1)
  S  = Zc Zc^T + eps I;  S /= ||S||_F
  B via 5 Newton-Schulz iters;  W = (B Zc) / sqrt(||S||_F)

Measured constraints this version is built around (v5/v6 traces):
  * PE 128-wide matmuls are LDWEIGHTS-paced (~70ns bf16, ~halved for
    1-byte operands since LDW time scales with stationary bytes), so
    on-device transposes (288 extra LDW ops + 18us of DVE copies) cost
    more than shipping Z^T from the host: it arrives pre-transposed in
    fp8-e4m3 with the rowsum ones-columns baked in ([128 data | 1.0]
    blocks, stride 129). Gram precision is unaffected: fp8 quantization
    noise averages over K=18432 (sim rel err 9.1e-3, gate 2e-2).
  * DMA sustains ~380 GB/s/core one-directional, ~270 mixed; input
    traffic is 14.2 MB (9.44 bf16 Z + 4.76 fp8 Z^T), output 9.44 MB,
    ordered zt(0), zt(1), z(0), z(1) so compute starts at ~3us and
    stores overlap the back half.
  * HAM throttles the PE 2x when duty drops: the schedule keeps PE fed
    (lazy mean-chain PE ops, NS woven into gram/proj, dead fp8 matmul
    fillers in copy-paced spans).
  * fp32-sourced output copies run 1x and are the proj-phase floor
    (~21us across DVE+ACT); they are 1024 wide (2-bank PSUM tiles).
"""

import math
from contextlib import ExitStack

import numpy as np
from ml_dtypes import bfloat16, float8_e4m3fn

import concourse.bacc as bacc
import concourse.mybir as mybir
from concourse.bass import ds, ts, MemorySpace
from concourse.bass_isa import ReduceOp
from concourse.bass_utils import run_bass_kernel_spmd
from concourse.tile import TileContext

P = 128
K = 18432
G_TOTAL = 16
N_CORES = 8
G_PER_CORE = G_TOTAL // N_CORES
ROWS_PER_CORE = G_PER_CORE * P
T_NS = 5
EPS = 1e-5

BF16 = mybir.dt.bfloat16
F8 = mybir.dt.float8e4
F32 = mybir.dt.float32

N_BLK = K // P              # 144 gram blocks per group
BLKW = P + 1                # 129: data + ones column
KT = N_BLK * BLKW           # 18576 fp8 zt columns per group
ZT_CHUNK = 48 * BLKW        # 6192: 6KB descriptors, 3 loads per group
LOAD_CHUNK = 6144           # 12KB descriptors, 3 loads per group
PROJ_U = 1024               # proj psum unit (2 banks)
N_UNITS = K // PROJ_U       # 18 per group
OUT_CHUNK = 2048            # store granularity (2 units)


def build_nc():
    nc = bacc.Bacc("TRN2", target_bir_lowering=False)
    x = nc.dram_tensor("x", [ROWS_PER_CORE, K], BF16, kind="ExternalInput")
    xt = nc.dram_tensor("xt", [ROWS_PER_CORE, KT], F8, kind="ExternalInput")
    y = nc.dram_tensor("y", [ROWS_PER_CORE, K], BF16, kind="ExternalOutput")

    with TileContext(nc) as tc, ExitStack() as ctx:
        consts = ctx.enter_context(tc.tile_pool(name="consts", bufs=1))

        def diag_const(dtype, fill, tag):
            t = consts.tile([P, P], dtype, tag=tag, name=tag)
            nc.gpsimd.memset(t, 0.0)
            nc.gpsimd.affine_select(
                out=t, in_=t, compare_op=mybir.AluOpType.not_equal,
                fill=fill, base=0, pattern=[[-1, P]], channel_multiplier=1,
            )
            return t

        id_bf = diag_const(BF16, 1.0, "idbf")
        eye15 = diag_const(BF16, 1.5, "eye15")
        eps_eye = diag_const(F32, EPS, "epseye")
        ones_bf = consts.tile([P, P], BF16, tag="onesbf")
        nc.gpsimd.memset(ones_bf, 1.0)

        ztp = ctx.enter_context(tc.tile_pool(name="zt", bufs=2 * 3))
        zpool = ctx.enter_context(tc.tile_pool(name="z", bufs=2 * 3))
        outp = ctx.enter_context(tc.tile_pool(name="out", bufs=4))
        nsp = ctx.enter_context(tc.tile_pool(name="ns", bufs=2))
        vecp = ctx.enter_context(tc.tile_pool(name="vec", bufs=2))
        ps_S = ctx.enter_context(tc.tile_pool(name="psS", bufs=2, space=MemorySpace.PSUM))
        ps_ns = ctx.enter_context(tc.tile_pool(name="psN", bufs=2, space=MemorySpace.PSUM))
        ps_proj = ctx.enter_context(tc.tile_pool(name="psP", bufs=2, space=MemorySpace.PSUM))

        st = [dict() for _ in range(G_PER_CORE)]

        def emit_zt_loads(g, split_first=False):
            s = st[g]
            s["zts"] = []
            for c in range(3):
                zt = ztp.tile([P, ZT_CHUNK], F8, tag="zt8", name=f"zt{g}_{c}")
                if split_first and c == 0:
                    for t3 in range(3):
                        nc.sync.dma_start(
                            zt[:, ts(t3, ZT_CHUNK // 3)],
                            xt[ds(g * P, P),
                               ds(t3 * (ZT_CHUNK // 3), ZT_CHUNK // 3)],
                        )
                else:
                    nc.sync.dma_start(zt, xt[ds(g * P, P), ts(c, ZT_CHUNK)])
                s["zts"].append(zt)

        def emit_z_loads(g):
            s = st[g]
            s["zs"] = []
            for c in range(3):
                z = zpool.tile([P, LOAD_CHUNK], BF16, tag="z", name=f"z{g}_{c}")
                nc.sync.dma_start(z, x[ds(g * P, P), ts(c, LOAD_CHUNK)])
                s["zs"].append(z)

        def zt_block(g, b, w):
            c, o = divmod(b * BLKW, ZT_CHUNK)
            return st[g]["zts"][c][:, ds(o, w)]

        def z_cols(g, col, width):
            c, o = divmod(col, LOAD_CHUNK)
            assert o + width <= LOAD_CHUNK
            return st[g]["zs"][c][:, ds(o, width)]

        def emit_gram_block(g, b):
            s = st[g]
            if b == 0:
                s["S_ps"] = ps_S.tile([P, P + 1], F32, tag="S", name=f"Sps{g}")
            nc.tensor.matmul(
                s["S_ps"], zt_block(g, b, P), zt_block(g, b, P + 1),
                start=(b == 0), stop=False,
            )

        _fill = [0]

        def emit_filler(n):
            """Dead fp8 matmuls (group 1 zt blocks) to hold HAM boost."""
            dead = ps_S.tile([P, P + 1], F32, tag="S", name=f"dead{_fill[0]}")
            for i in range(n):
                b = (_fill[0] * 7 + i * 13) % N_BLK
                nc.tensor.matmul(
                    dead, zt_block(1, b, P), zt_block(1, b, P + 1),
                    start=(i == 0), stop=(i == n - 1),
                )
            _fill[0] += 1

        def emit_mean_A(g):
            s = st[g]
            r = vecp.tile([P, 1], F32, tag=f"r{g}", name=f"r{g}")
            nc.vector.tensor_copy(r, s["S_ps"][:, ds(P, 1)])
            s["r"] = r
            m12 = vecp.tile([P, 1], F32, tag=f"m12{g}", name=f"m12{g}")
            nc.vector.tensor_scalar_mul(m12, r, math.sqrt(K / P) / K)
            Mm = vecp.tile([P, P], BF16, tag=f"Mm{g}", name=f"Mm{g}")
            nc.vector.tensor_scalar_mul(Mm, ones_bf, m12)
            s["Mm"] = Mm

        def emit_mean_B(g):
            s = st[g]
            M_ps = ps_ns.tile([P, P], BF16, tag="ns", name=f"Mps{g}")
            nc.tensor.transpose(M_ps, s["Mm"], id_bf)
            Ma = vecp.tile([P, P + 1], BF16, tag=f"Ma{g}", name=f"Ma{g}")
            nc.gpsimd.memset(Ma[:, ds(P, 1)], 0.0)
            nc.vector.tensor_copy(Ma[:, ds(0, P)], M_ps)
            Mb = vecp.tile([P, P], BF16, tag=f"Mb{g}", name=f"Mb{g}")
            nc.vector.tensor_scalar_mul(Mb, Ma[:, ds(0, P)], -1.0)
            nc.tensor.matmul(s["S_ps"], Mb, Ma, start=False, stop=True)

        def emit_mean_C(g):
            s = st[g]
            S = nsp.tile([P, P], F32, tag="S", name=f"S{g}")
            nc.vector.scalar_tensor_tensor(
                S, s["S_ps"][:, ds(0, P)], 1.0, eps_eye,
                mybir.AluOpType.mult, mybir.AluOpType.add,
            )
            sq = nsp.tile([P, P], BF16, tag="sq", name=f"sq{g}")
            frob2 = vecp.tile([P, 1], F32, tag=f"fr{g}", name=f"fr{g}")
            nc.scalar.activation(
                sq, S, mybir.ActivationFunctionType.Square, accum_out=frob2
            )
            nc.gpsimd.partition_all_reduce(frob2, frob2, P, ReduceOp.add)
            nu = vecp.tile([P, 1], F32, tag=f"nu{g}", name=f"nu{g}")
            nc.scalar.sqrt(nu, frob2)
            nu2 = vecp.tile([P, 1], F32, tag=f"nu2{g}", name=f"nu2{g}")
            nc.vector.tensor_scalar_mul(nu2, nu, 2.0)
            inv_nu2 = vecp.tile([P, 1], F32, tag=f"inu{g}", name=f"inu{g}")
            nc.vector.reciprocal(inv_nu2, nu2)          # 1/(2 nu)
            inv_nu = vecp.tile([P, 1], F32, tag=f"invnu{g}", name=f"invnu{g}")
            nc.vector.tensor_scalar_mul(inv_nu, inv_nu2, 2.0)
            osc = vecp.tile([P, 1], F32, tag=f"osc{g}", name=f"osc{g}")
            nc.scalar.activation(osc, inv_nu, mybir.ActivationFunctionType.Sqrt)
            s["osc"] = osc
            Sh = nsp.tile([P, P], BF16, tag="Sh", name=f"Sh{g}")
            nc.scalar.activation(
                Sh, S, mybir.ActivationFunctionType.Identity, scale=inv_nu2
            )
            s["Sh"] = Sh
            B = nsp.tile([P, P], BF16, tag=f"B{g}", name=f"B0_{g}")
            nc.vector.tensor_sub(B, eye15, Sh)
            s["B"] = B

        def emit_ns_step(g, it, sub):
            s = st[g]
            if sub == 0:
                bb_ps = ps_ns.tile([P, P], F32, tag="ns", name=f"bb{g}_{it}")
                nc.tensor.matmul(bb_ps, s["B"], s["B"], start=True, stop=True)
                BB = nsp.tile([P, P], BF16, tag=f"BB{g}", name=f"BB{g}_{it}")
                nc.vector.tensor_copy(BB, bb_ps)
                s["BB"] = BB
            elif sub == 1:
                b3_ps = ps_ns.tile([P, P], F32, tag="ns", name=f"b3{g}_{it}")
                nc.tensor.matmul(b3_ps, s["BB"], s["B"], start=True, stop=True)
                B3 = nsp.tile([P, P], BF16, tag=f"B3{g}", name=f"B3_{g}_{it}")
                nc.vector.tensor_copy(B3, b3_ps)
                s["B3"] = B3
            else:
                p_ps = ps_ns.tile([P, P], F32, tag="ns", name=f"pp{g}_{it}")
                nc.tensor.matmul(p_ps, s["B3"], s["Sh"], start=True, stop=True)
                Bn = nsp.tile([P, P], BF16, tag=f"Bn{g}", name=f"Bn{g}_{it}")
                nc.vector.scalar_tensor_tensor(
                    Bn, s["B"], 1.5, p_ps,
                    mybir.AluOpType.mult, mybir.AluOpType.subtract,
                )
                s["B"] = Bn

        def emit_cbias(g):
            s = st[g]
            Bs = nsp.tile([P, P], BF16, tag=f"Bs{g}", name=f"Bs{g}")
            nc.vector.tensor_scalar_mul(Bs, s["B"], s["osc"])
            s["Bs"] = Bs
            mu = vecp.tile([P, 1], BF16, tag=f"mu{g}", name=f"mu{g}")
            nc.vector.tensor_scalar_mul(mu, s["r"], 1.0 / K)
            c_ps = ps_ns.tile([P, 1], F32, tag="ns", name=f"cps{g}")
            nc.tensor.matmul(c_ps, Bs, mu, start=True, stop=True)
            nbias = vecp.tile([P, 1], F32, tag=f"nb{g}", name=f"nb{g}")
            nc.vector.tensor_scalar_mul(nbias, c_ps, -1.0)
            s["nbias"] = nbias

        def emit_proj_unit(g, u, eng):
            s = st[g]
            c, t = divmod(u, OUT_CHUNK // PROJ_U)
            if t == 0:
                s["out_t"] = outp.tile(
                    [P, OUT_CHUNK], BF16, tag="out", name=f"o{g}_{c}"
                )
            pr = ps_proj.tile([P, PROJ_U], F32, tag="pr", name=f"pr{g}_{u}")
            for h in range(2):
                nc.tensor.matmul(
                    pr[:, ts(h, 512)], s["Bs"],
                    z_cols(g, u * PROJ_U + h * 512, 512),
                    start=True, stop=True,
                )
            dst = s["out_t"][:, ts(t, PROJ_U)]
            if eng == "d":
                nc.vector.tensor_scalar_add(dst, pr, s["nbias"])
            else:
                nc.scalar.activation(
                    dst, pr, mybir.ActivationFunctionType.Identity,
                    bias=s["nbias"],
                )
            if t == OUT_CHUNK // PROJ_U - 1:
                nc.sync.dma_start(y[ds(g * P, P), ts(c, OUT_CHUNK)], s["out_t"])

        # ---------------- emission schedule ----------------
        emit_zt_loads(0, split_first=True)
        emit_zt_loads(1)
        emit_z_loads(0)
        emit_z_loads(1)

        # B: gram(0) solid
        for b in range(N_BLK):
            emit_gram_block(0, b)
        emit_mean_A(0)

        # C: gram(1) woven with mean_B/C(0) and NS(0)
        ns_steps = [(it, sub) for it in range(T_NS - 1) for sub in range(3)]
        k = 0
        for b in range(N_BLK):
            emit_gram_block(1, b)
            if b == 8:
                emit_mean_B(0)
            elif b == 16:
                emit_mean_C(0)
            elif b > 24:
                want = (b - 24) * len(ns_steps) // (N_BLK - 25)
                while k < want:
                    emit_ns_step(0, *ns_steps[k])
                    k += 1
        while k < len(ns_steps):
            emit_ns_step(0, *ns_steps[k])
            emit_filler(2)
            k += 1
        emit_cbias(0)

        # E: proj(0) units 0..11 with mean(1) woven in + light filler
        oc = "dadadadadada"
        for u in range(12):
            emit_proj_unit(0, u, oc[u])
            emit_filler(1)
            if u == 8:
                emit_mean_A(1)
            elif u == 9:
                emit_mean_B(1)
            elif u == 10:
                emit_mean_C(1)

        # F: proj(0) units 12..17 (copies mostly ACT), NS(1) on DVE
        k = 0
        oc = "aadaad"
        for i, u in enumerate(range(12, 18)):
            want = (i + 1) * len(ns_steps) // 6
            while k < want:
                emit_ns_step(1, *ns_steps[k])
                k += 1
            emit_proj_unit(0, u, oc[i])
            emit_filler(1)
        while k < len(ns_steps):
            emit_ns_step(1, *ns_steps[k])
            emit_filler(2)
            k += 1
        emit_cbias(1)

        # G: proj(1) with boost filler
        oc = "dadadadadadadadada"
        for u in range(N_UNITS):
            emit_proj_unit(1, u, oc[u])
            emit_filler(2)

    nc.finalize()
    return nc


_NC_CACHE = None


def _get_nc():
    global _NC_CACHE
    if _NC_CACHE is None:
        _NC_CACHE = build_nc()
    return _NC_CACHE


def _host_prep(w):
    """Build bf16 Z and fp8 Z^T-with-ones streams."""
    wb = w.astype(bfloat16)
    zt = np.ones((G_TOTAL, P, N_BLK, BLKW), dtype=np.float32)
    blocks = w.reshape(G_TOTAL, P, N_BLK, P)        # [g, r, b, k]
    zt[:, :, :, :P] = blocks.transpose(0, 3, 2, 1)  # [g, k, b, r]
    zt8 = zt.reshape(G_TOTAL, P, KT).astype(float8_e4m3fn)
    return wb, zt8


def kernel(weight, _trace=False):
    w = np.ascontiguousarray(np.asarray(weight, dtype=np.float32))
    assert w.shape == (G_TOTAL * P, K), w.shape
    wb, zt8 = _host_prep(w)
    nc = _get_nc()
    in_maps = []
    for core in range(N_CORES):
        in_maps.append({
            "x": np.ascontiguousarray(
                wb[core * ROWS_PER_CORE:(core + 1) * ROWS_PER_CORE]),
            "xt": np.ascontiguousarray(
                zt8[core * G_PER_CORE:(core + 1) * G_PER_CORE].reshape(
                    ROWS_PER_CORE, KT)),
        })
    res = run_bass_kernel_spmd(
        nc, in_maps, core_ids=list(range(N_CORES)), trace=_trace
    )
    out = np.concatenate(
        [np.asarray(r["y"]).astype(np.float32) for r in res.results], axis=0
    )
    if _trace:
        return out, res
    return out


# revision 11
# speedup vs baseline: 1.2689x; 1.0329x over previous
"""ONI-Norm TRN2 kernel v7: fp8 host-transposed Gram stream + bf16 proj.

Math per group g (128 rows, fan_in K=18432):
  Zc = Z - mean(Z, axis=1)
  S  = Zc Zc^T + eps I;  S /= ||S||_F
  B via 5 Newton-Schulz iters;  W = (B Zc) / sqrt(||S||_F)

Measured constraints this version is built around (v5/v6 traces):
  * PE 128-wide matmuls are LDWEIGHTS-paced (~70ns bf16, ~halved for
    1-byte operands since LDW time scales with stationary bytes), so
    on-device transposes (288 extra LDW ops + 18us of DVE copies) cost
    more than shipping Z^T from the host: it arrives pre-transposed in
    fp8-e4m3 with the rowsum ones-columns baked in ([128 data | 1.0]
    blocks, stride 129). Gram precision is unaffected: fp8 quantization
    noise averages over K=18432 (sim rel err 9.1e-3, gate 2e-2).
  * DMA sustains ~380 GB/s/core one-directional, ~270 mixed; input
    traffic is 14.2 MB (9.44 bf16 Z + 4.76 fp8 Z^T), output 9.44 MB,
    ordered zt(0), zt(1), z(0), z(1) so compute starts at ~3us and
    stores overlap the back half.
  * HAM throttles the PE 2x when duty drops: the schedule keeps PE fed
    (lazy mean-chain PE ops, NS woven into gram/proj, dead fp8 matmul
    fillers in copy-paced spans).
  * fp32-sourced output copies run 1x and are the proj-phase floor
    (~21us across DVE+ACT); they are 1024 wide (2-bank PSUM tiles).
"""

import math
from contextlib import ExitStack

import numpy as np
from ml_dtypes import bfloat16, float8_e4m3fn

import concourse.bacc as bacc
import concourse.mybir as mybir
from concourse.bass import ds, ts, MemorySpace
from concourse.bass_isa import ReduceOp
from concourse.bass_utils import run_bass_kernel_spmd
from concourse.tile import TileContext

P = 128
K = 18432
G_TOTAL = 16
N_CORES = 8
G_PER_CORE = G_TOTAL // N_CORES
ROWS_PER_CORE = G_PER_CORE * P
T_NS = 5
EPS = 1e-5

BF16 = mybir.dt.bfloat16
F8 = mybir.dt.float8e4
F32 = mybir.dt.float32

N_BLK = K // P              # 144 gram blocks per group
BLKW = P + 1                # 129: data + ones column
KT = N_BLK * BLKW           # 18576 fp8 zt columns per group
ZT_CHUNK = 48 * BLKW        # 6192: 6KB descriptors, 3 loads per group
LOAD_CHUNK = 6144           # 12KB descriptors, 3 loads per group
PROJ_U = 1024               # proj psum unit (2 banks)
N_UNITS = K // PROJ_U       # 18 per group
OUT_CHUNK = 2048            # store granularity (2 units)


def build_nc():
    nc = bacc.Bacc("TRN2", target_bir_lowering=False)
    x = nc.dram_tensor("x", [ROWS_PER_CORE, K], BF16, kind="ExternalInput")
    xt = nc.dram_tensor("xt", [ROWS_PER_CORE, KT], F8, kind="ExternalInput")
    y = nc.dram_tensor("y", [ROWS_PER_CORE, K], BF16, kind="ExternalOutput")

    with TileContext(nc) as tc, ExitStack() as ctx:
        consts = ctx.enter_context(tc.tile_pool(name="consts", bufs=1))

        def diag_const(dtype, fill, tag):
            t = consts.tile([P, P], dtype, tag=tag, name=tag)
            nc.gpsimd.memset(t, 0.0)
            nc.gpsimd.affine_select(
                out=t, in_=t, compare_op=mybir.AluOpType.not_equal,
                fill=fill, base=0, pattern=[[-1, P]], channel_multiplier=1,
            )
            return t

        id_bf = diag_const(BF16, 1.0, "idbf")
        eye15 = diag_const(BF16, 1.5, "eye15")
        eps_eye = diag_const(F32, EPS, "epseye")
        ones_bf = consts.tile([P, P], BF16, tag="onesbf")
        nc.gpsimd.memset(ones_bf, 1.0)

        ztp = ctx.enter_context(tc.tile_pool(name="zt", bufs=2 * 3))
        zpool = ctx.enter_context(tc.tile_pool(name="z", bufs=2 * 3))
        outp = ctx.enter_context(tc.tile_pool(name="out", bufs=4))
        nsp = ctx.enter_context(tc.tile_pool(name="ns", bufs=2))
        vecp = ctx.enter_context(tc.tile_pool(name="vec", bufs=2))
        ps_S = ctx.enter_context(tc.tile_pool(name="psS", bufs=2, space=MemorySpace.PSUM))
        ps_ns = ctx.enter_context(tc.tile_pool(name="psN", bufs=2, space=MemorySpace.PSUM))
        ps_proj = ctx.enter_context(tc.tile_pool(name="psP", bufs=2, space=MemorySpace.PSUM))

        st = [dict() for _ in range(G_PER_CORE)]

        def emit_zt_loads(g, split_first=False):
            s = st[g]
            s["zts"] = []
            for c in range(3):
                zt = ztp.tile([P, ZT_CHUNK], F8, tag="zt8", name=f"zt{g}_{c}")
                if split_first and c == 0:
                    for t3 in range(3):
                        nc.sync.dma_start(
                            zt[:, ts(t3, ZT_CHUNK // 3)],
                            xt[ds(g * P, P),
                               ds(t3 * (ZT_CHUNK // 3), ZT_CHUNK // 3)],
                        )
                else:
                    nc.sync.dma_start(zt, xt[ds(g * P, P), ts(c, ZT_CHUNK)])
                s["zts"].append(zt)

        def emit_z_loads(g):
            s = st[g]
            s["zs"] = []
            for c in range(3):
                z = zpool.tile([P, LOAD_CHUNK], BF16, tag="z", name=f"z{g}_{c}")
                nc.sync.dma_start(z, x[ds(g * P, P), ts(c, LOAD_CHUNK)])
                s["zs"].append(z)

        def zt_block(g, b, w):
            c, o = divmod(b * BLKW, ZT_CHUNK)
            return st[g]["zts"][c][:, ds(o, w)]

        def z_cols(g, col, width):
            c, o = divmod(col, LOAD_CHUNK)
            assert o + width <= LOAD_CHUNK
            return st[g]["zs"][c][:, ds(o, width)]

        def emit_gram_block(g, b):
            s = st[g]
            if b == 0:
                s["S_ps"] = ps_S.tile([P, P + 1], F32, tag="S", name=f"Sps{g}")
            nc.tensor.matmul(
                s["S_ps"], zt_block(g, b, P), zt_block(g, b, P + 1),
                start=(b == 0), stop=False,
            )

        _fill = [0]

        def emit_filler(n):
            """Dead fp8 matmuls (group 1 zt blocks) to hold HAM boost."""
            dead = ps_S.tile([P, P + 1], F32, tag="S", name=f"dead{_fill[0]}")
            for i in range(n):
                b = (_fill[0] * 7 + i * 13) % N_BLK
                nc.tensor.matmul(
                    dead, zt_block(1, b, P), zt_block(1, b, P + 1),
                    start=(i == 0), stop=(i == n - 1),
                )
            _fill[0] += 1

        def emit_mean_A(g):
            s = st[g]
            r = vecp.tile([P, 1], F32, tag=f"r{g}", name=f"r{g}")
            nc.vector.tensor_copy(r, s["S_ps"][:, ds(P, 1)])
            s["r"] = r
            m12 = vecp.tile([P, 1], F32, tag=f"m12{g}", name=f"m12{g}")
            nc.vector.tensor_scalar_mul(m12, r, math.sqrt(K / P) / K)
            Mm = vecp.tile([P, P], BF16, tag=f"Mm{g}", name=f"Mm{g}")
            nc.vector.tensor_scalar_mul(Mm, ones_bf, m12)
            s["Mm"] = Mm

        def emit_mean_B(g):
            s = st[g]
            M_ps = ps_ns.tile([P, P], BF16, tag="ns", name=f"Mps{g}")
            nc.tensor.transpose(M_ps, s["Mm"], id_bf)
            Ma = vecp.tile([P, P + 1], BF16, tag=f"Ma{g}", name=f"Ma{g}")
            nc.gpsimd.memset(Ma[:, ds(P, 1)], 0.0)
            nc.vector.tensor_copy(Ma[:, ds(0, P)], M_ps)
            Mb = vecp.tile([P, P], BF16, tag=f"Mb{g}", name=f"Mb{g}")
            nc.vector.tensor_scalar_mul(Mb, Ma[:, ds(0, P)], -1.0)
            nc.tensor.matmul(s["S_ps"], Mb, Ma, start=False, stop=True)

        def emit_mean_C(g):
            s = st[g]
            S = nsp.tile([P, P], F32, tag="S", name=f"S{g}")
            nc.vector.scalar_tensor_tensor(
                S, s["S_ps"][:, ds(0, P)], 1.0, eps_eye,
                mybir.AluOpType.mult, mybir.AluOpType.add,
            )
            sq = nsp.tile([P, P], BF16, tag="sq", name=f"sq{g}")
            frob2 = vecp.tile([P, 1], F32, tag=f"fr{g}", name=f"fr{g}")
            nc.scalar.activation(
                sq, S, mybir.ActivationFunctionType.Square, accum_out=frob2
            )
            nc.gpsimd.partition_all_reduce(frob2, frob2, P, ReduceOp.add)
            nu = vecp.tile([P, 1], F32, tag=f"nu{g}", name=f"nu{g}")
            nc.scalar.sqrt(nu, frob2)
            nu2 = vecp.tile([P, 1], F32, tag=f"nu2{g}", name=f"nu2{g}")
            nc.vector.tensor_scalar_mul(nu2, nu, 2.0)
            inv_nu2 = vecp.tile([P, 1], F32, tag=f"inu{g}", name=f"inu{g}")
            nc.vector.reciprocal(inv_nu2, nu2)          # 1/(2 nu)
            inv_nu = vecp.tile([P, 1], F32, tag=f"invnu{g}", name=f"invnu{g}")
            nc.vector.tensor_scalar_mul(inv_nu, inv_nu2, 2.0)
            osc = vecp.tile([P, 1], F32, tag=f"osc{g}", name=f"osc{g}")
            nc.scalar.activation(osc, inv_nu, mybir.ActivationFunctionType.Sqrt)
            s["osc"] = osc
            Sh = nsp.tile([P, P], BF16, tag="Sh", name=f"Sh{g}")
            nc.scalar.activation(
                Sh, S, mybir.ActivationFunctionType.Identity, scale=inv_nu2
            )
            s["Sh"] = Sh
            B = nsp.tile([P, P], BF16, tag=f"B{g}", name=f"B0_{g}")
            nc.vector.tensor_sub(B, eye15, Sh)
            s["B"] = B

        def emit_ns_step(g, it, sub):
            s = st[g]
            if sub == 0:
                bb_ps = ps_ns.tile([P, P], F32, tag="ns", name=f"bb{g}_{it}")
                nc.tensor.matmul(bb_ps, s["B"], s["B"], start=True, stop=True)
                BB = nsp.tile([P, P], BF16, tag=f"BB{g}", name=f"BB{g}_{it}")
                nc.vector.tensor_copy(BB, bb_ps)
                s["BB"] = BB
            elif sub == 1:
                b3_ps = ps_ns.tile([P, P], F32, tag="ns", name=f"b3{g}_{it}")
                nc.tensor.matmul(b3_ps, s["BB"], s["B"], start=True, stop=True)
                B3 = nsp.tile([P, P], BF16, tag=f"B3{g}", name=f"B3_{g}_{it}")
                nc.vector.tensor_copy(B3, b3_ps)
                s["B3"] = B3
            else:
                p_ps = ps_ns.tile([P, P], F32, tag="ns", name=f"pp{g}_{it}")
                nc.tensor.matmul(p_ps, s["B3"], s["Sh"], start=True, stop=True)
                Bn = nsp.tile([P, P], BF16, tag=f"Bn{g}", name=f"Bn{g}_{it}")
                nc.vector.scalar_tensor_tensor(
                    Bn, s["B"], 1.5, p_ps,
                    mybir.AluOpType.mult, mybir.AluOpType.subtract,
                )
                s["B"] = Bn

        def emit_cbias(g):
            s = st[g]
            Bs = nsp.tile([P, P], BF16, tag=f"Bs{g}", name=f"Bs{g}")
            nc.vector.tensor_scalar_mul(Bs, s["B"], s["osc"])
            s["Bs"] = Bs
            mu = vecp.tile([P, 1], BF16, tag=f"mu{g}", name=f"mu{g}")
            nc.vector.tensor_scalar_mul(mu, s["r"], 1.0 / K)
            c_ps = ps_ns.tile([P, 1], F32, tag="ns", name=f"cps{g}")
            nc.tensor.matmul(c_ps, Bs, mu, start=True, stop=True)
            nbias = vecp.tile([P, 1], F32, tag=f"nb{g}", name=f"nb{g}")
            nc.vector.tensor_scalar_mul(nbias, c_ps, -1.0)
            s["nbias"] = nbias

        def emit_proj_unit(g, u, eng):
            s = st[g]
            c, t = divmod(u, OUT_CHUNK // PROJ_U)
            if t == 0:
                s["out_t"] = outp.tile(
                    [P, OUT_CHUNK], BF16, tag="out", name=f"o{g}_{c}"
                )
            pr = ps_proj.tile([P, PROJ_U], F32, tag="pr", name=f"pr{g}_{u}")
            for h in range(2):
                nc.tensor.matmul(
                    pr[:, ts(h, 512)], s["Bs"],
                    z_cols(g, u * PROJ_U + h * 512, 512),
                    start=True, stop=True,
                )
            dst = s["out_t"][:, ts(t, PROJ_U)]
            if eng == "d":
                nc.vector.tensor_scalar_add(dst, pr, s["nbias"])
            else:
                nc.scalar.activation(
                    dst, pr, mybir.ActivationFunctionType.Identity,
                    bias=s["nbias"],
                )
            if t == OUT_CHUNK // PROJ_U - 1:
                nc.sync.dma_start(y[ds(g * P, P), ts(c, OUT_CHUNK)], s["out_t"])

        # ---------------- emission schedule ----------------
        emit_zt_loads(0, split_first=True)
        emit_zt_loads(1)
        emit_z_loads(0)
        emit_z_loads(1)

        # B: gram(0) solid
        for b in range(N_BLK):
            emit_gram_block(0, b)
        emit_mean_A(0)

        # C: gram(1) woven with mean_B/C(0) and NS(0) (NS done by block ~128)
        ns_steps = [(it, sub) for it in range(T_NS - 1) for sub in range(3)]
        k = 0
        for b in range(N_BLK):
            emit_gram_block(1, b)
            if b == 8:
                emit_mean_B(0)
            elif b == 16:
                emit_mean_C(0)
            elif 24 < b <= 128:
                want = (b - 24) * len(ns_steps) // (128 - 24)
                while k < want:
                    emit_ns_step(0, *ns_steps[k])
                    k += 1
        while k < len(ns_steps):
            emit_ns_step(0, *ns_steps[k])
            emit_filler(2)
            k += 1
        emit_cbias(0)
        emit_mean_A(1)

        # E: proj(0) all 18 units; mean_B/C(1) + NS(1) + cbias(1) woven in
        k = 0
        oc = "adaadadaadadaadada"
        for u in range(N_UNITS):
            emit_proj_unit(0, u, oc[u])
            if u == 0:
                emit_mean_B(1)
            elif u == 1:
                emit_mean_C(1)
            elif 2 <= u <= 13:
                want = (u - 1) * len(ns_steps) // 12
                while k < want:
                    emit_ns_step(1, *ns_steps[k])
                    k += 1
            elif u == 14:
                while k < len(ns_steps):
                    emit_ns_step(1, *ns_steps[k])
                    k += 1
                emit_cbias(1)

        # G: proj(1) with boost filler
        oc = "adaadadaadadaadada"
        for u in range(N_UNITS):
            emit_proj_unit(1, u, oc[u])
            emit_filler(1)

    nc.finalize()
    return nc


_NC_CACHE = None


def _get_nc():
    global _NC_CACHE
    if _NC_CACHE is None:
        _NC_CACHE = build_nc()
    return _NC_CACHE


def _host_prep(w):
    """Build bf16 Z and fp8 Z^T-with-ones streams."""
    wb = w.astype(bfloat16)
    zt = np.ones((G_TOTAL, P, N_BLK, BLKW), dtype=np.float32)
    blocks = w.reshape(G_TOTAL, P, N_BLK, P)        # [g, r, b, k]
    zt[:, :, :, :P] = blocks.transpose(0, 3, 2, 1)  # [g, k, b, r]
    zt8 = zt.reshape(G_TOTAL, P, KT).astype(float8_e4m3fn)
    return wb, zt8


def kernel(weight, _trace=False):
    w = np.ascontiguousarray(np.asarray(weight, dtype=np.float32))
    assert w.shape == (G_TOTAL * P, K), w.shape
    wb, zt8 = _host_prep(w)
    nc = _get_nc()
    in_maps = []
    for core in range(N_CORES):
        in_maps.append({
            "x": np.ascontiguousarray(
                wb[core * ROWS_PER_CORE:(core + 1) * ROWS_PER_CORE]),
            "xt": np.ascontiguousarray(
                zt8[core * G_PER_CORE:(core + 1) * G_PER_CORE].reshape(
                    ROWS_PER_CORE, KT)),
        })
    res = run_bass_kernel_spmd(
        nc, in_maps, core_ids=list(range(N_CORES)), trace=_trace
    )
    out = np.concatenate(
        [np.asarray(r["y"]).astype(np.float32) for r in res.results], axis=0
    )
    if _trace:
        return out, res
    return out


# revision 12
# speedup vs baseline: 1.2928x; 1.0188x over previous
"""ONI-Norm TRN2 kernel v7: fp8 host-transposed Gram stream + bf16 proj.

Math per group g (128 rows, fan_in K=18432):
  Zc = Z - mean(Z, axis=1)
  S  = Zc Zc^T + eps I;  S /= ||S||_F
  B via 5 Newton-Schulz iters;  W = (B Zc) / sqrt(||S||_F)

Measured constraints this version is built around (v5/v6 traces):
  * PE 128-wide matmuls are LDWEIGHTS-paced (~70ns bf16, ~halved for
    1-byte operands since LDW time scales with stationary bytes), so
    on-device transposes (288 extra LDW ops + 18us of DVE copies) cost
    more than shipping Z^T from the host: it arrives pre-transposed in
    fp8-e4m3 with the rowsum ones-columns baked in ([128 data | 1.0]
    blocks, stride 129). Gram precision is unaffected: fp8 quantization
    noise averages over K=18432 (sim rel err 9.1e-3, gate 2e-2).
  * DMA sustains ~380 GB/s/core one-directional, ~270 mixed; input
    traffic is 14.2 MB (9.44 bf16 Z + 4.76 fp8 Z^T), output 9.44 MB,
    ordered zt(0), zt(1), z(0), z(1) so compute starts at ~3us and
    stores overlap the back half.
  * HAM throttles the PE 2x when duty drops: the schedule keeps PE fed
    (lazy mean-chain PE ops, NS woven into gram/proj, dead fp8 matmul
    fillers in copy-paced spans).
  * fp32-sourced output copies run 1x and are the proj-phase floor
    (~21us across DVE+ACT); they are 1024 wide (2-bank PSUM tiles).
"""

import math
from contextlib import ExitStack

import numpy as np
from ml_dtypes import bfloat16, float8_e4m3fn

import concourse.bacc as bacc
import concourse.mybir as mybir
from concourse.bass import ds, ts, MemorySpace
from concourse.bass_isa import ReduceOp
from concourse.bass_utils import run_bass_kernel_spmd
from concourse.tile import TileContext

P = 128
K = 18432
G_TOTAL = 16
N_CORES = 8
G_PER_CORE = G_TOTAL // N_CORES
ROWS_PER_CORE = G_PER_CORE * P
T_NS = 5
EPS = 1e-5

BF16 = mybir.dt.bfloat16
F8 = mybir.dt.float8e4
F32 = mybir.dt.float32

N_BLK = K // P              # 144 gram blocks per group
BLKW = P + 1                # 129: data + ones column
KT = N_BLK * BLKW           # 18576 fp8 zt columns per group
ZT_CHUNK = 48 * BLKW        # 6192: 6KB descriptors, 3 loads per group
LOAD_CHUNK = 6144           # 12KB descriptors, 3 loads per group
PROJ_U = 1024               # proj psum unit (2 banks)
N_UNITS = K // PROJ_U       # 18 per group
OUT_CHUNK = 2048            # store granularity (2 units)


def build_nc():
    nc = bacc.Bacc("TRN2", target_bir_lowering=False)
    x = nc.dram_tensor("x", [ROWS_PER_CORE, K], BF16, kind="ExternalInput")
    xt = nc.dram_tensor("xt", [ROWS_PER_CORE, KT], F8, kind="ExternalInput")
    y = nc.dram_tensor("y", [ROWS_PER_CORE, K], BF16, kind="ExternalOutput")

    with TileContext(nc) as tc, ExitStack() as ctx:
        consts = ctx.enter_context(tc.tile_pool(name="consts", bufs=1))

        def diag_const(dtype, fill, tag):
            t = consts.tile([P, P], dtype, tag=tag, name=tag)
            nc.gpsimd.memset(t, 0.0)
            nc.gpsimd.affine_select(
                out=t, in_=t, compare_op=mybir.AluOpType.not_equal,
                fill=fill, base=0, pattern=[[-1, P]], channel_multiplier=1,
            )
            return t

        id_bf = diag_const(BF16, 1.0, "idbf")
        eye15 = diag_const(BF16, 1.5, "eye15")
        eps_eye = diag_const(F32, EPS, "epseye")
        ones_bf = consts.tile([P, P], BF16, tag="onesbf")
        nc.gpsimd.memset(ones_bf, 1.0)

        ztp = ctx.enter_context(tc.tile_pool(name="zt", bufs=2 * 3))
        zpool = ctx.enter_context(tc.tile_pool(name="z", bufs=2 * 3))
        outp = ctx.enter_context(tc.tile_pool(name="out", bufs=4))
        nsp = ctx.enter_context(tc.tile_pool(name="ns", bufs=2))
        vecp = ctx.enter_context(tc.tile_pool(name="vec", bufs=2))
        ps_S = ctx.enter_context(tc.tile_pool(name="psS", bufs=2, space=MemorySpace.PSUM))
        ps_ns = ctx.enter_context(tc.tile_pool(name="psN", bufs=2, space=MemorySpace.PSUM))
        ps_proj = ctx.enter_context(tc.tile_pool(name="psP", bufs=2, space=MemorySpace.PSUM))

        st = [dict() for _ in range(G_PER_CORE)]

        def emit_zt_loads(g, split_first=False):
            s = st[g]
            s["zts"] = []
            for c in range(3):
                zt = ztp.tile([P, ZT_CHUNK], F8, tag="zt8", name=f"zt{g}_{c}")
                if split_first and c == 0:
                    for t3 in range(3):
                        nc.sync.dma_start(
                            zt[:, ts(t3, ZT_CHUNK // 3)],
                            xt[ds(g * P, P),
                               ds(t3 * (ZT_CHUNK // 3), ZT_CHUNK // 3)],
                        )
                else:
                    nc.sync.dma_start(zt, xt[ds(g * P, P), ts(c, ZT_CHUNK)])
                s["zts"].append(zt)

        def emit_z_loads(g):
            s = st[g]
            s["zs"] = []
            for c in range(3):
                z = zpool.tile([P, LOAD_CHUNK], BF16, tag="z", name=f"z{g}_{c}")
                nc.sync.dma_start(z, x[ds(g * P, P), ts(c, LOAD_CHUNK)])
                s["zs"].append(z)

        def zt_block(g, b, w):
            c, o = divmod(b * BLKW, ZT_CHUNK)
            return st[g]["zts"][c][:, ds(o, w)]

        def z_cols(g, col, width):
            c, o = divmod(col, LOAD_CHUNK)
            assert o + width <= LOAD_CHUNK
            return st[g]["zs"][c][:, ds(o, width)]

        def emit_gram_block(g, b):
            s = st[g]
            if b == 0:
                s["S_ps"] = ps_S.tile([P, P + 1], F32, tag="S", name=f"Sps{g}")
            nc.tensor.matmul(
                s["S_ps"], zt_block(g, b, P), zt_block(g, b, P + 1),
                start=(b == 0), stop=False,
            )

        _fill = [0]

        def emit_filler(n):
            """Dead fp8 matmuls (group 1 zt blocks) to hold HAM boost."""
            dead = ps_S.tile([P, P + 1], F32, tag="S", name=f"dead{_fill[0]}")
            for i in range(n):
                b = (_fill[0] * 7 + i * 13) % N_BLK
                nc.tensor.matmul(
                    dead, zt_block(1, b, P), zt_block(1, b, P + 1),
                    start=(i == 0), stop=(i == n - 1),
                )
            _fill[0] += 1

        def emit_mean_A(g):
            s = st[g]
            r = vecp.tile([P, 1], F32, tag=f"r{g}", name=f"r{g}")
            nc.vector.tensor_copy(r, s["S_ps"][:, ds(P, 1)])
            s["r"] = r
            m12 = vecp.tile([P, 1], F32, tag=f"m12{g}", name=f"m12{g}")
            nc.vector.tensor_scalar_mul(m12, r, math.sqrt(K / P) / K)
            Mm = vecp.tile([P, P], BF16, tag=f"Mm{g}", name=f"Mm{g}")
            nc.vector.tensor_scalar_mul(Mm, ones_bf, m12)
            s["Mm"] = Mm

        def emit_mean_B(g):
            s = st[g]
            M_ps = ps_ns.tile([P, P], BF16, tag="ns", name=f"Mps{g}")
            nc.tensor.transpose(M_ps, s["Mm"], id_bf)
            Ma = vecp.tile([P, P + 1], BF16, tag=f"Ma{g}", name=f"Ma{g}")
            nc.gpsimd.memset(Ma[:, ds(P, 1)], 0.0)
            nc.vector.tensor_copy(Ma[:, ds(0, P)], M_ps)
            Mb = vecp.tile([P, P], BF16, tag=f"Mb{g}", name=f"Mb{g}")
            nc.vector.tensor_scalar_mul(Mb, Ma[:, ds(0, P)], -1.0)
            nc.tensor.matmul(s["S_ps"], Mb, Ma, start=False, stop=True)

        def emit_mean_C(g):
            s = st[g]
            S = nsp.tile([P, P], F32, tag="S", name=f"S{g}")
            nc.vector.scalar_tensor_tensor(
                S, s["S_ps"][:, ds(0, P)], 1.0, eps_eye,
                mybir.AluOpType.mult, mybir.AluOpType.add,
            )
            sq = nsp.tile([P, P], BF16, tag="sq", name=f"sq{g}")
            frob2 = vecp.tile([P, 1], F32, tag=f"fr{g}", name=f"fr{g}")
            nc.scalar.activation(
                sq, S, mybir.ActivationFunctionType.Square, accum_out=frob2
            )
            nc.gpsimd.partition_all_reduce(frob2, frob2, P, ReduceOp.add)
            nu = vecp.tile([P, 1], F32, tag=f"nu{g}", name=f"nu{g}")
            nc.scalar.sqrt(nu, frob2)
            nu2 = vecp.tile([P, 1], F32, tag=f"nu2{g}", name=f"nu2{g}")
            nc.vector.tensor_scalar_mul(nu2, nu, 2.0)
            inv_nu2 = vecp.tile([P, 1], F32, tag=f"inu{g}", name=f"inu{g}")
            nc.vector.reciprocal(inv_nu2, nu2)          # 1/(2 nu)
            inv_nu = vecp.tile([P, 1], F32, tag=f"invnu{g}", name=f"invnu{g}")
            nc.vector.tensor_scalar_mul(inv_nu, inv_nu2, 2.0)
            osc = vecp.tile([P, 1], F32, tag=f"osc{g}", name=f"osc{g}")
            nc.scalar.activation(osc, inv_nu, mybir.ActivationFunctionType.Sqrt)
            s["osc"] = osc
            Sh = nsp.tile([P, P], BF16, tag="Sh", name=f"Sh{g}")
            nc.scalar.activation(
                Sh, S, mybir.ActivationFunctionType.Identity, scale=inv_nu2
            )
            s["Sh"] = Sh
            B = nsp.tile([P, P], BF16, tag=f"B{g}", name=f"B0_{g}")
            nc.vector.tensor_sub(B, eye15, Sh)
            s["B"] = B

        def emit_ns_step(g, it, sub):
            s = st[g]
            if sub == 0:
                bb_ps = ps_ns.tile([P, P], F32, tag="ns", name=f"bb{g}_{it}")
                nc.tensor.matmul(bb_ps, s["B"], s["B"], start=True, stop=True)
                BB = nsp.tile([P, P], BF16, tag=f"BB{g}", name=f"BB{g}_{it}")
                nc.vector.tensor_copy(BB, bb_ps)
                s["BB"] = BB
            elif sub == 1:
                b3_ps = ps_ns.tile([P, P], F32, tag="ns", name=f"b3{g}_{it}")
                nc.tensor.matmul(b3_ps, s["BB"], s["B"], start=True, stop=True)
                B3 = nsp.tile([P, P], BF16, tag=f"B3{g}", name=f"B3_{g}_{it}")
                nc.vector.tensor_copy(B3, b3_ps)
                s["B3"] = B3
            else:
                p_ps = ps_ns.tile([P, P], F32, tag="ns", name=f"pp{g}_{it}")
                nc.tensor.matmul(p_ps, s["B3"], s["Sh"], start=True, stop=True)
                Bn = nsp.tile([P, P], BF16, tag=f"Bn{g}", name=f"Bn{g}_{it}")
                nc.vector.scalar_tensor_tensor(
                    Bn, s["B"], 1.5, p_ps,
                    mybir.AluOpType.mult, mybir.AluOpType.subtract,
                )
                s["B"] = Bn

        def emit_cbias(g):
            s = st[g]
            Bs = nsp.tile([P, P], BF16, tag=f"Bs{g}", name=f"Bs{g}")
            nc.vector.tensor_scalar_mul(Bs, s["B"], s["osc"])
            s["Bs"] = Bs
            mu = vecp.tile([P, 1], BF16, tag=f"mu{g}", name=f"mu{g}")
            nc.vector.tensor_scalar_mul(mu, s["r"], 1.0 / K)
            c_ps = ps_ns.tile([P, 1], F32, tag="ns", name=f"cps{g}")
            nc.tensor.matmul(c_ps, Bs, mu, start=True, stop=True)
            nbias = vecp.tile([P, 1], F32, tag=f"nb{g}", name=f"nb{g}")
            nc.vector.tensor_scalar_mul(nbias, c_ps, -1.0)
            s["nbias"] = nbias

        def emit_proj_unit(g, u, eng):
            s = st[g]
            c, t = divmod(u, OUT_CHUNK // PROJ_U)
            if t == 0:
                s["out_t"] = outp.tile(
                    [P, OUT_CHUNK], BF16, tag="out", name=f"o{g}_{c}"
                )
            pr = ps_proj.tile([P, PROJ_U], F32, tag="pr", name=f"pr{g}_{u}")
            for h in range(2):
                nc.tensor.matmul(
                    pr[:, ts(h, 512)], s["Bs"],
                    z_cols(g, u * PROJ_U + h * 512, 512),
                    start=True, stop=True,
                )
            dst = s["out_t"][:, ts(t, PROJ_U)]
            if eng == "d":
                nc.vector.tensor_scalar_add(dst, pr, s["nbias"])
            else:
                nc.scalar.activation(
                    dst, pr, mybir.ActivationFunctionType.Identity,
                    bias=s["nbias"],
                )
            if t == OUT_CHUNK // PROJ_U - 1:
                nc.sync.dma_start(y[ds(g * P, P), ts(c, OUT_CHUNK)], s["out_t"])

        # ---------------- emission schedule ----------------
        emit_zt_loads(0, split_first=True)
        emit_zt_loads(1)
        st[0]["zs"] = []
        st[1]["zs"] = []
        for c in range(3):
            for g in range(2):
                z = zpool.tile([P, LOAD_CHUNK], BF16, tag="z", name=f"z{g}_{c}")
                nc.sync.dma_start(z, x[ds(g * P, P), ts(c, LOAD_CHUNK)])
                st[g]["zs"].append(z)

        # B: gram(0) solid
        for b in range(N_BLK):
            emit_gram_block(0, b)
        emit_mean_A(0)

        # C: gram(1) woven with mean_B/C(0) and NS(0) (NS done by block ~128)
        ns_steps = [(it, sub) for it in range(T_NS - 1) for sub in range(3)]
        k = 0
        for b in range(N_BLK):
            emit_gram_block(1, b)
            if b == 8:
                emit_mean_B(0)
            elif b == 16:
                emit_mean_C(0)
            elif 24 < b <= 128:
                want = (b - 24) * len(ns_steps) // (128 - 24)
                while k < want:
                    emit_ns_step(0, *ns_steps[k])
                    k += 1
        while k < len(ns_steps):
            emit_ns_step(0, *ns_steps[k])
            emit_filler(2)
            k += 1
        emit_cbias(0)
        emit_mean_A(1)

        # E: proj(0) all 18 units; mean_B/C(1) + NS(1) + cbias(1) woven in
        k = 0
        oc = "adaadadaadadaadada"
        for u in range(N_UNITS):
            emit_proj_unit(0, u, oc[u])
            if u >= 2:
                emit_filler(2)
            if u == 0:
                emit_mean_B(1)
            elif u == 1:
                emit_mean_C(1)
            elif 2 <= u <= 13:
                want = (u - 1) * len(ns_steps) // 12
                while k < want:
                    emit_ns_step(1, *ns_steps[k])
                    k += 1
            elif u == 14:
                while k < len(ns_steps):
                    emit_ns_step(1, *ns_steps[k])
                    k += 1
                emit_cbias(1)

        # G: proj(1) with boost filler
        oc = "adaadadaadadaadada"
        for u in range(N_UNITS):
            emit_proj_unit(1, u, oc[u])
            emit_filler(3)

    nc.finalize()
    return nc


_NC_CACHE = None


def _get_nc():
    global _NC_CACHE
    if _NC_CACHE is None:
        _NC_CACHE = build_nc()
    return _NC_CACHE


def _host_prep(w):
    """Build bf16 Z and fp8 Z^T-with-ones streams."""
    wb = w.astype(bfloat16)
    zt = np.ones((G_TOTAL, P, N_BLK, BLKW), dtype=np.float32)
    blocks = w.reshape(G_TOTAL, P, N_BLK, P)        # [g, r, b, k]
    zt[:, :, :, :P] = blocks.transpose(0, 3, 2, 1)  # [g, k, b, r]
    zt8 = zt.reshape(G_TOTAL, P, KT).astype(float8_e4m3fn)
    return wb, zt8


def kernel(weight, _trace=False):
    w = np.ascontiguousarray(np.asarray(weight, dtype=np.float32))
    assert w.shape == (G_TOTAL * P, K), w.shape
    wb, zt8 = _host_prep(w)
    nc = _get_nc()
    in_maps = []
    for core in range(N_CORES):
        in_maps.append({
            "x": np.ascontiguousarray(
                wb[core * ROWS_PER_CORE:(core + 1) * ROWS_PER_CORE]),
            "xt": np.ascontiguousarray(
                zt8[core * G_PER_CORE:(core + 1) * G_PER_CORE].reshape(
                    ROWS_PER_CORE, KT)),
        })
    res = run_bass_kernel_spmd(
        nc, in_maps, core_ids=list(range(N_CORES)), trace=_trace
    )
    out = np.concatenate(
        [np.asarray(r["y"]).astype(np.float32) for r in res.results], axis=0
    )
    if _trace:
        return out, res
    return out


# revision 13
# speedup vs baseline: 1.4753x; 1.1412x over previous
"""ONI-Norm TRN2 kernel v7: fp8 host-transposed Gram stream + bf16 proj.

Math per group g (128 rows, fan_in K=18432):
  Zc = Z - mean(Z, axis=1)
  S  = Zc Zc^T + eps I;  S /= ||S||_F
  B via 5 Newton-Schulz iters;  W = (B Zc) / sqrt(||S||_F)

Measured constraints this version is built around (v5/v6 traces):
  * PE 128-wide matmuls are LDWEIGHTS-paced (~70ns bf16, ~halved for
    1-byte operands since LDW time scales with stationary bytes), so
    on-device transposes (288 extra LDW ops + 18us of DVE copies) cost
    more than shipping Z^T from the host: it arrives pre-transposed in
    fp8-e4m3 with the rowsum ones-columns baked in ([128 data | 1.0]
    blocks, stride 129). Gram precision is unaffected: fp8 quantization
    noise averages over K=18432 (sim rel err 9.1e-3, gate 2e-2).
  * DMA sustains ~380 GB/s/core one-directional, ~270 mixed; input
    traffic is 14.2 MB (9.44 bf16 Z + 4.76 fp8 Z^T), output 9.44 MB,
    ordered zt(0), zt(1), z(0), z(1) so compute starts at ~3us and
    stores overlap the back half.
  * HAM throttles the PE 2x when duty drops: the schedule keeps PE fed
    (lazy mean-chain PE ops, NS woven into gram/proj, dead fp8 matmul
    fillers in copy-paced spans).
  * fp32-sourced output copies run 1x and are the proj-phase floor
    (~21us across DVE+ACT); they are 1024 wide (2-bank PSUM tiles).
"""

import math
from contextlib import ExitStack

import numpy as np
from ml_dtypes import bfloat16, float8_e4m3fn

import concourse.bacc as bacc
import concourse.mybir as mybir
from concourse.bass import ds, ts, MemorySpace
from concourse.bass_isa import ReduceOp
from concourse.bass_utils import run_bass_kernel_spmd
from concourse.tile import TileContext

P = 128
K = 18432
G_TOTAL = 16
N_CORES = 8
G_PER_CORE = G_TOTAL // N_CORES
ROWS_PER_CORE = G_PER_CORE * P
T_NS = 5
EPS = 1e-5

BF16 = mybir.dt.bfloat16
F8 = mybir.dt.float8e4
F32 = mybir.dt.float32

N_BLK = K // P              # 144 gram blocks per group
BLKW = P + 1                # 129: data + ones column
KT = N_BLK * BLKW           # 18576 fp8 zt columns per group
ZT_CHUNK = 48 * BLKW        # 6192: 6KB descriptors, 3 loads per group
LOAD_CHUNK = 6144           # 12KB descriptors, 3 loads per group
PROJ_U = 512                # proj psum unit (1 bank, 4-deep pipeline)
N_UNITS = K // PROJ_U       # 36 per group
OUT_CHUNK = 2048            # store granularity (4 units)


def build_nc():
    nc = bacc.Bacc("TRN2", target_bir_lowering=False)
    x = nc.dram_tensor("x", [ROWS_PER_CORE, K], BF16, kind="ExternalInput")
    xt = nc.dram_tensor("xt", [ROWS_PER_CORE, KT], F8, kind="ExternalInput")
    y = nc.dram_tensor("y", [ROWS_PER_CORE, K], BF16, kind="ExternalOutput")

    with TileContext(nc) as tc, ExitStack() as ctx:
        consts = ctx.enter_context(tc.tile_pool(name="consts", bufs=1))

        def diag_const(dtype, fill, tag):
            t = consts.tile([P, P], dtype, tag=tag, name=tag)
            nc.gpsimd.memset(t, 0.0)
            nc.gpsimd.affine_select(
                out=t, in_=t, compare_op=mybir.AluOpType.not_equal,
                fill=fill, base=0, pattern=[[-1, P]], channel_multiplier=1,
            )
            return t

        id_bf = diag_const(BF16, 1.0, "idbf")
        eye15 = diag_const(BF16, 1.5, "eye15")
        eps_eye = diag_const(F32, EPS, "epseye")
        ones_bf = consts.tile([P, P], BF16, tag="onesbf")
        nc.gpsimd.memset(ones_bf, 1.0)

        ztp = ctx.enter_context(tc.tile_pool(name="zt", bufs=2 * 3))
        zpool = ctx.enter_context(tc.tile_pool(name="z", bufs=2 * 3))
        outp = ctx.enter_context(tc.tile_pool(name="out", bufs=4))
        nsp = ctx.enter_context(tc.tile_pool(name="ns", bufs=2))
        vecp = ctx.enter_context(tc.tile_pool(name="vec", bufs=2))
        ps_S = ctx.enter_context(tc.tile_pool(name="psS", bufs=2, space=MemorySpace.PSUM))
        ps_ns = ctx.enter_context(tc.tile_pool(name="psN", bufs=2, space=MemorySpace.PSUM))
        ps_proj = ctx.enter_context(tc.tile_pool(name="psP", bufs=4, space=MemorySpace.PSUM))

        st = [dict() for _ in range(G_PER_CORE)]

        def emit_zt_loads(g, split_first=False):
            s = st[g]
            s["zts"] = []
            for c in range(3):
                zt = ztp.tile([P, ZT_CHUNK], F8, tag="zt8", name=f"zt{g}_{c}")
                if split_first and c == 0:
                    for t3 in range(3):
                        nc.sync.dma_start(
                            zt[:, ts(t3, ZT_CHUNK // 3)],
                            xt[ds(g * P, P),
                               ds(t3 * (ZT_CHUNK // 3), ZT_CHUNK // 3)],
                        )
                else:
                    nc.sync.dma_start(zt, xt[ds(g * P, P), ts(c, ZT_CHUNK)])
                s["zts"].append(zt)

        def emit_z_loads(g):
            s = st[g]
            s["zs"] = []
            for c in range(3):
                z = zpool.tile([P, LOAD_CHUNK], BF16, tag="z", name=f"z{g}_{c}")
                nc.sync.dma_start(z, x[ds(g * P, P), ts(c, LOAD_CHUNK)])
                s["zs"].append(z)

        def zt_block(g, b, w):
            c, o = divmod(b * BLKW, ZT_CHUNK)
            return st[g]["zts"][c][:, ds(o, w)]

        def z_cols(g, col, width):
            c, o = divmod(col, LOAD_CHUNK)
            assert o + width <= LOAD_CHUNK
            return st[g]["zs"][c][:, ds(o, width)]

        def emit_gram_block(g, b):
            s = st[g]
            if b == 0:
                s["S_ps"] = ps_S.tile([P, P + 1], F32, tag="S", name=f"Sps{g}")
            nc.tensor.matmul(
                s["S_ps"], zt_block(g, b, P), zt_block(g, b, P + 1),
                start=(b == 0), stop=False,
            )

        _fill = [0]

        def emit_filler(n):
            """Dead fp8 matmuls (group 1 zt blocks) to hold HAM boost."""
            dead = ps_S.tile([P, P + 1], F32, tag="S", name=f"dead{_fill[0]}")
            for i in range(n):
                b = (_fill[0] * 7 + i * 13) % N_BLK
                nc.tensor.matmul(
                    dead, zt_block(1, b, P), zt_block(1, b, P + 1),
                    start=(i == 0), stop=(i == n - 1),
                )
            _fill[0] += 1

        def emit_mean_A(g):
            s = st[g]
            r = vecp.tile([P, 1], F32, tag=f"r{g}", name=f"r{g}")
            nc.vector.tensor_copy(r, s["S_ps"][:, ds(P, 1)])
            s["r"] = r
            m12 = vecp.tile([P, 1], F32, tag=f"m12{g}", name=f"m12{g}")
            nc.vector.tensor_scalar_mul(m12, r, math.sqrt(K / P) / K)
            Mm = vecp.tile([P, P], BF16, tag=f"Mm{g}", name=f"Mm{g}")
            nc.vector.tensor_scalar_mul(Mm, ones_bf, m12)
            s["Mm"] = Mm

        def emit_mean_B(g):
            s = st[g]
            M_ps = ps_ns.tile([P, P], BF16, tag="ns", name=f"Mps{g}")
            nc.tensor.transpose(M_ps, s["Mm"], id_bf)
            Ma = vecp.tile([P, P + 1], BF16, tag=f"Ma{g}", name=f"Ma{g}")
            nc.gpsimd.memset(Ma[:, ds(P, 1)], 0.0)
            nc.vector.tensor_copy(Ma[:, ds(0, P)], M_ps)
            Mb = vecp.tile([P, P], BF16, tag=f"Mb{g}", name=f"Mb{g}")
            nc.vector.tensor_scalar_mul(Mb, Ma[:, ds(0, P)], -1.0)
            nc.tensor.matmul(s["S_ps"], Mb, Ma, start=False, stop=True)

        def emit_mean_C(g):
            s = st[g]
            S = nsp.tile([P, P], F32, tag="S", name=f"S{g}")
            nc.vector.scalar_tensor_tensor(
                S, s["S_ps"][:, ds(0, P)], 1.0, eps_eye,
                mybir.AluOpType.mult, mybir.AluOpType.add,
            )
            sq = nsp.tile([P, P], BF16, tag="sq", name=f"sq{g}")
            frob2 = vecp.tile([P, 1], F32, tag=f"fr{g}", name=f"fr{g}")
            nc.scalar.activation(
                sq, S, mybir.ActivationFunctionType.Square, accum_out=frob2
            )
            nc.gpsimd.partition_all_reduce(frob2, frob2, P, ReduceOp.add)
            nu = vecp.tile([P, 1], F32, tag=f"nu{g}", name=f"nu{g}")
            nc.scalar.sqrt(nu, frob2)
            nu2 = vecp.tile([P, 1], F32, tag=f"nu2{g}", name=f"nu2{g}")
            nc.vector.tensor_scalar_mul(nu2, nu, 2.0)
            inv_nu2 = vecp.tile([P, 1], F32, tag=f"inu{g}", name=f"inu{g}")
            nc.vector.reciprocal(inv_nu2, nu2)          # 1/(2 nu)
            inv_nu = vecp.tile([P, 1], F32, tag=f"invnu{g}", name=f"invnu{g}")
            nc.vector.tensor_scalar_mul(inv_nu, inv_nu2, 2.0)
            osc = vecp.tile([P, 1], F32, tag=f"osc{g}", name=f"osc{g}")
            nc.scalar.activation(osc, inv_nu, mybir.ActivationFunctionType.Sqrt)
            s["osc"] = osc
            Sh = nsp.tile([P, P], BF16, tag="Sh", name=f"Sh{g}")
            nc.scalar.activation(
                Sh, S, mybir.ActivationFunctionType.Identity, scale=inv_nu2
            )
            s["Sh"] = Sh
            B = nsp.tile([P, P], BF16, tag=f"B{g}", name=f"B0_{g}")
            nc.vector.tensor_sub(B, eye15, Sh)
            s["B"] = B

        def emit_ns_step(g, it, sub):
            s = st[g]
            if sub == 0:
                bb_ps = ps_ns.tile([P, P], F32, tag="ns", name=f"bb{g}_{it}")
                nc.tensor.matmul(bb_ps, s["B"], s["B"], start=True, stop=True)
                BB = nsp.tile([P, P], BF16, tag=f"BB{g}", name=f"BB{g}_{it}")
                nc.vector.tensor_copy(BB, bb_ps)
                s["BB"] = BB
            elif sub == 1:
                b3_ps = ps_ns.tile([P, P], F32, tag="ns", name=f"b3{g}_{it}")
                nc.tensor.matmul(b3_ps, s["BB"], s["B"], start=True, stop=True)
                B3 = nsp.tile([P, P], BF16, tag=f"B3{g}", name=f"B3_{g}_{it}")
                nc.vector.tensor_copy(B3, b3_ps)
                s["B3"] = B3
            else:
                p_ps = ps_ns.tile([P, P], F32, tag="ns", name=f"pp{g}_{it}")
                nc.tensor.matmul(p_ps, s["B3"], s["Sh"], start=True, stop=True)
                Bn = nsp.tile([P, P], BF16, tag=f"Bn{g}", name=f"Bn{g}_{it}")
                nc.vector.scalar_tensor_tensor(
                    Bn, s["B"], 1.5, p_ps,
                    mybir.AluOpType.mult, mybir.AluOpType.subtract,
                )
                s["B"] = Bn

        def emit_cbias(g):
            s = st[g]
            Bs = nsp.tile([P, P], BF16, tag=f"Bs{g}", name=f"Bs{g}")
            nc.vector.tensor_scalar_mul(Bs, s["B"], s["osc"])
            s["Bs"] = Bs
            mu = vecp.tile([P, 1], BF16, tag=f"mu{g}", name=f"mu{g}")
            nc.vector.tensor_scalar_mul(mu, s["r"], 1.0 / K)
            c_ps = ps_ns.tile([P, 1], F32, tag="ns", name=f"cps{g}")
            nc.tensor.matmul(c_ps, Bs, mu, start=True, stop=True)
            nbias = vecp.tile([P, 1], F32, tag=f"nb{g}", name=f"nb{g}")
            nc.vector.tensor_scalar_mul(nbias, c_ps, -1.0)
            s["nbias"] = nbias

        def emit_proj_unit(g, u, eng):
            s = st[g]
            c, t = divmod(u, OUT_CHUNK // PROJ_U)
            if t == 0:
                s["out_t"] = outp.tile(
                    [P, OUT_CHUNK], BF16, tag="out", name=f"o{g}_{c}"
                )
            pr = ps_proj.tile([P, PROJ_U], F32, tag="pr", name=f"pr{g}_{u}")
            nc.tensor.matmul(
                pr, s["Bs"], z_cols(g, u * PROJ_U, PROJ_U),
                start=True, stop=True,
            )
            dst = s["out_t"][:, ts(t, PROJ_U)]
            if eng == "d":
                nc.vector.tensor_scalar_add(dst, pr, s["nbias"])
            else:
                nc.scalar.activation(
                    dst, pr, mybir.ActivationFunctionType.Identity,
                    bias=s["nbias"],
                )
            if t == OUT_CHUNK // PROJ_U - 1:
                nc.sync.dma_start(y[ds(g * P, P), ts(c, OUT_CHUNK)], s["out_t"])

        # ---------------- emission schedule ----------------
        emit_zt_loads(0, split_first=True)
        emit_zt_loads(1)
        st[0]["zs"] = []
        st[1]["zs"] = []
        for c in range(3):
            for g in range(2):
                z = zpool.tile([P, LOAD_CHUNK], BF16, tag="z", name=f"z{g}_{c}")
                nc.sync.dma_start(z, x[ds(g * P, P), ts(c, LOAD_CHUNK)])
                st[g]["zs"].append(z)

        # B: gram(0) solid
        for b in range(N_BLK):
            emit_gram_block(0, b)
        emit_mean_A(0)

        # C: gram(1) woven with mean_B/C(0) and NS(0) (NS done by block ~128)
        ns_steps = [(it, sub) for it in range(T_NS - 1) for sub in range(3)]
        k = 0
        for b in range(N_BLK):
            emit_gram_block(1, b)
            if b == 8:
                emit_mean_B(0)
            elif b == 16:
                emit_mean_C(0)
            elif 24 < b <= 128:
                want = (b - 24) * len(ns_steps) // (128 - 24)
                while k < want:
                    emit_ns_step(0, *ns_steps[k])
                    k += 1
        while k < len(ns_steps):
            emit_ns_step(0, *ns_steps[k])
            emit_filler(2)
            k += 1
        emit_cbias(0)
        emit_mean_A(1)

        # E: proj(0) all 36 units; mean_B/C(1) + NS(1) + cbias(1) woven in
        k = 0
        for u in range(N_UNITS):
            emit_proj_unit(0, u, "ad"[u % 2])
            if u % 2 == 0:
                emit_filler(1)
            if u == 0:
                emit_mean_B(1)
            elif u == 1:
                emit_mean_C(1)
            elif 4 <= u <= 27:
                want = (u - 3) * len(ns_steps) // 24
                while k < want:
                    emit_ns_step(1, *ns_steps[k])
                    k += 1
            elif u == 28:
                while k < len(ns_steps):
                    emit_ns_step(1, *ns_steps[k])
                    k += 1
                emit_cbias(1)

        # G: proj(1) with boost filler
        for u in range(N_UNITS):
            emit_proj_unit(1, u, "ad"[u % 2])
            if u % 2 == 0:
                emit_filler(2)

    nc.finalize()
    return nc


_NC_CACHE = None


def _get_nc():
    global _NC_CACHE
    if _NC_CACHE is None:
        _NC_CACHE = build_nc()
    return _NC_CACHE


def _host_prep(w):
    """Build bf16 Z and fp8 Z^T-with-ones streams."""
    wb = w.astype(bfloat16)
    zt = np.ones((G_TOTAL, P, N_BLK, BLKW), dtype=np.float32)
    blocks = w.reshape(G_TOTAL, P, N_BLK, P)        # [g, r, b, k]
    zt[:, :, :, :P] = blocks.transpose(0, 3, 2, 1)  # [g, k, b, r]
    zt8 = zt.reshape(G_TOTAL, P, KT).astype(float8_e4m3fn)
    return wb, zt8


def kernel(weight, _trace=False):
    w = np.ascontiguousarray(np.asarray(weight, dtype=np.float32))
    assert w.shape == (G_TOTAL * P, K), w.shape
    wb, zt8 = _host_prep(w)
    nc = _get_nc()
    in_maps = []
    for core in range(N_CORES):
        in_maps.append({
            "x": np.ascontiguousarray(
                wb[core * ROWS_PER_CORE:(core + 1) * ROWS_PER_CORE]),
            "xt": np.ascontiguousarray(
                zt8[core * G_PER_CORE:(core + 1) * G_PER_CORE].reshape(
                    ROWS_PER_CORE, KT)),
        })
    res = run_bass_kernel_spmd(
        nc, in_maps, core_ids=list(range(N_CORES)), trace=_trace
    )
    out = np.concatenate(
        [np.asarray(r["y"]).astype(np.float32) for r in res.results], axis=0
    )
    if _trace:
        return out, res
    return out
